# revision 45
# baseline (speedup 1.0000x reference)
"""BERT self-attention (B=4, S=2048, H=1024, 16 heads x 64) on 8 TRN2 NeuronCores.

Sharding: data-parallel over batch (4) x tensor-parallel over head-groups (2).
Core c handles batch c//2 and heads [8*(c%2), 8*(c%2)+8): it gets the full
hidden_states[b] plus the 512 W-columns/bias entries for its heads, and
produces out[b, :, 512*g : 512*(g+1)]. No cross-core communication.

Per-core kernel (bf16/fp16 matmuls, f32 accumulation in PSUM):
  xT   = transpose(x) via PE (bf16)               [1024h, 2048s]
  QT/KT = W.T @ xT  (+bias)                       [512hd, 2048s]
  V'   = xT.T @ Wv (+bias), 65 cols per head with an appended ones column
  per (head-pair, q-macro 512, k-chunk 128):
    scoresT[k, q] = KT_h[:, kc].T @ QT_h[:, qm]   (two heads row-packed, K=64)
    expT = exp(0.125 * scoresT)                   (ACT, fp16 out, N=1024/inst)
    ctxT[65, q] += V'_h[kc].T @ expT              (row 64 = softmax denominator)
    + one Q/K projection matmul of the NEXT head-pair (fills the PE gap
      while ACT paces the loop at ~1.1us/k-chunk)
  epilogue: ctxT -> hardware DMA-transpose -> [q, 65]; multiply by
  reciprocal denominator (GPSIMD); batched DMA out via HWDGE.

Schedule notes (from perfetto traces):
 - ACT exp (FD=1024 from PSUM) is the loop pacer at ~1.07-1.11us/iter;
   the PE instruction stream (ctx pair + row-packed score pair + one
   proj MM) is ~1.08us/iter, so both engines run ~100% in steady state.
   PSUM's 8 banks are exactly spent (2x2 score double-buffer + 2 ctx
   accumulators + 2 proj accumulators), which caps the exp FD at 1024.
 - The epilogue is split: the PSUM-draining copies + DMA-transposes are
   issued at the period boundary, but the reciprocal/scale/store half is
   deferred one full period so its DVE ops sit after the next period's
   proj bias-adds in the DVE FIFO and can never head-of-line-block them
   (that stall idled ACT ~1.7us per boundary and HAM-re-throttled PE).
 - The last ctx of a period can only run after the last exp, which
   starves the PE through the turnover (the 2-slot score ring can't run
   ahead). Chunks kc=1..4's proj+scores are priority-hoisted to the
   period start so the boundary window stays full; the ctx backlog
   drains through the 10-deep exp-tile ring. Hoisting more chunks
   back-fires (priority ties scramble the score-ring order).
 - All staging DMAs go through the sync-engine HWDGE rings (SWDGE
   DIRECT2D enqueues cost ~640ns each on the GPSIMD engine), with x
   prefetched ahead of the wq/wk staging (DMA rings round-robin, so
   enqueue order decides arrival order under bandwidth contention).
Measured on TRN2: ~372us HW exec (baseline 412us), rel l2 err ~3.7e-3.
"""

import sys
import types

sys.path.insert(0, "/opt/trn_rl_repo")

import numpy as np

import concourse.bass as bass
import concourse.tile as tile
from concourse import bacc, mybir
from concourse.bass_utils import run_bass_kernel_spmd
from concourse.masks import make_identity

B, S, H = 4, 2048, 1024
NH, HD = 16, 64
NCORES = 8
HEADS_PER_CORE = NH // 2      # 8 heads per core
HG = HEADS_PER_CORE * HD      # 512 = per-core head width
P = 128
QM = 512                      # q macro-tile
N_QM = S // QM                # 4
N_KC = S // P                 # 16 k chunks
N_ST = S // P                 # 16 s tiles
N_HB = H // P                 # 8 h chunks (contraction)
N_MT = HG // P                # 4 hd m-tiles

FP32 = mybir.dt.float32
BF16 = mybir.dt.bfloat16
FP16 = mybir.dt.float16


def _ensure_profile_hook():
    """The image's antenv lacks axon_hooks; shim it so trace=True works."""
    try:
        from antenv.axon_hooks import get_axon_ntff_profile_hook  # noqa: F401
        return
    except ImportError:
        pass
    try:
        from trn_agent_boot.trn_boot import _ntff_profile_via_ctypes
    except ImportError:
        return
    hook = _ntff_profile_via_ctypes("/opt/axon/libaxon_pjrt.so")
    mod = types.ModuleType("antenv.axon_hooks")
    mod.get_axon_ntff_profile_hook = lambda: hook
    mod.set_axon_ntff_profile_hook = lambda h: None
    sys.modules["antenv.axon_hooks"] = mod


def build():
    nc = bacc.Bacc("TRN2", target_bir_lowering=False, debug=False,
                   num_devices=NCORES)

    x_d = nc.declare_dram_parameter("x", [S, H], FP32, isOutput=False)
    wq_d = nc.declare_dram_parameter("wq", [H, HG], FP32, isOutput=False)
    wk_d = nc.declare_dram_parameter("wk", [H, HG], FP32, isOutput=False)
    wv_d = nc.declare_dram_parameter("wv", [H, HG], FP32, isOutput=False)
    bq_d = nc.declare_dram_parameter("bq", [HG], FP32, isOutput=False)
    bk_d = nc.declare_dram_parameter("bk", [HG], FP32, isOutput=False)
    bv_d = nc.declare_dram_parameter("bv", [HG], FP32, isOutput=False)
    out_d = nc.declare_dram_parameter("out", [S, HG], FP32, isOutput=True)

    with tile.TileContext(nc) as tc:
        _build_body(nc, tc, x_d, (wq_d, wk_d, wv_d), (bq_d, bk_d, bv_d), out_d)

    nc.finalize()
    return nc


def _build_body(nc, tc, x_d, w_d, b_d, out_d):
    wq_d, wk_d, wv_d = w_d
    bq_d, bk_d, bv_d = b_d

    import contextlib
    from contextlib import nullcontext as _nullcontext
    ctx = contextlib.ExitStack()
    with ctx:
        const = ctx.enter_context(tc.tile_pool(name="const", bufs=1))
        xf = ctx.enter_context(tc.tile_pool(name="xf", bufs=4))
        xbp = ctx.enter_context(tc.tile_pool(name="xbp", bufs=3))
        big = ctx.enter_context(tc.tile_pool(name="big", bufs=1))
        wstage = ctx.enter_context(tc.tile_pool(name="wstage", bufs=3))
        expp = ctx.enter_context(tc.tile_pool(name="expp", bufs=10))
        epil = ctx.enter_context(tc.tile_pool(name="epil", bufs=4))
        outp = ctx.enter_context(tc.tile_pool(name="outp", bufs=4))
        # PSUM budget (8 banks): ps_sc = 2 x 2-bank slots (scores double
        # buffer), ps_ctx = 2 x 1-bank slots (ctx accumulators), ps_pj =
        # 2 x 1-bank slots (V'/QK projection accumulators).
        ps_sc = ctx.enter_context(
            tc.tile_pool(name="ps_sc", bufs=2, space="PSUM"))
        ps_ctx = ctx.enter_context(
            tc.tile_pool(name="ps_ctx", bufs=2, space="PSUM"))
        ps_pj = ctx.enter_context(
            tc.tile_pool(name="ps_pj", bufs=2, space="PSUM"))

        # ---- x prefetch first so the first s-tiles land ASAP -----------
        x_tiles = {}

        def fetch_x(st):
            xt = xf.tile([P, H], FP32, tag="x", name=f"x{st}")
            nc.sync.dma_start(out=xt, in_=x_d.ap()[st * P:(st + 1) * P, :])
            x_tiles[st] = xt

        for st in range(4):
            fetch_x(st)

        # ---- constants -------------------------------------------------
        ident_b128 = const.tile([P, P], BF16)
        make_identity(nc, ident_b128)
        ident_h = const.tile([HD + 1, HD + 1], FP16)
        make_identity(nc, ident_h)
        ident_b = ident_h
        bqT = const.tile([P, N_MT], FP32)
        nc.sync.dma_start(out=bqT, in_=bq_d.ap().rearrange("(o p) -> p o", p=P))
        bkT = const.tile([P, N_MT], FP32)
        nc.sync.dma_start(out=bkT, in_=bk_d.ap().rearrange("(o p) -> p o", p=P))
        bv_ap = bv_d.ap()
        bvb = const.tile([P, HG], FP32)
        nc.sync.dma_start(
            out=bvb,
            in_=bass.AP(tensor=bv_ap.tensor, offset=bv_ap.offset,
                        ap=[[0, P]] + [list(a) for a in bv_ap.ap]),
        )

        # ---- weights: staged via sync HWDGE, cast on DVE ---------------
        w_sb = {}
        for name, wd in (("q", wq_d), ("k", wk_d), ("v", wv_d)):
            w_sb[name] = big.tile([P, N_HB, HG], BF16, tag=f"w{name}",
                                  name=f"w{name}")

        def load_w(name, wd, ks):
            # casts on GPSIMD (idle in phase1): on DVE they sit ahead of
            # the xT-copies in the FIFO while waiting for the staging DMA,
            # which stalls the transpose PSUM ring (head-of-line block)
            for k in ks:
                stg = wstage.tile([P, HG], FP32, tag="wstg", name=f"w{name}{k}")
                nc.sync.dma_start(out=stg, in_=wd.ap()[k * P:(k + 1) * P, :])
                nc.gpsimd.tensor_copy(out=w_sb[name][:, k, :], in_=stg)

        load_w("v", wv_d, range(N_HB))

        # ---- per s-tile: load x, transpose to xT, project V' -----------
        xT = big.tile([P, N_HB, S], BF16, tag="xT")
        vp = big.tile([P, N_ST, HEADS_PER_CORE, HD + 1], BF16, tag="vp")
        # only the appended ones-column needs initializing; cols 0:HD are
        # fully written by the V' bias add below
        nc.vector.memset(vp[:, :, :, HD:HD + 1], 1.0)

        qT = big.tile([P, N_MT, S], BF16, tag="qT")
        kT = big.tile([P, N_MT, S], BF16, tag="kT")

        def proj_chunk(mt, n, pool=None, tag=None):
            pool = pool or ps_pj
            tag = tag or "pj"
            for w_name, dst, bias in (("q", qT, bqT), ("k", kT, bkT)):
                ps = pool.tile([P, QM], FP32, tag=tag,
                               name=f"proj{w_name}{mt}{n}")
                for k in range(N_HB):
                    nc.tensor.matmul(
                        ps,
                        lhsT=w_sb[w_name][:, k, mt * P:(mt + 1) * P],
                        rhs=xT[:, k, n * QM:(n + 1) * QM],
                        start=(k == 0),
                        stop=(k == N_HB - 1),
                    )
                nc.vector.tensor_scalar_add(
                    out=dst[:, mt, n * QM:(n + 1) * QM],
                    in0=ps,
                    scalar1=bias[:, mt:mt + 1],
                )

        def vprime(st):
            psv = ps_pj.tile([P, HG], FP32, tag="pj", name=f"v{st}")
            for hb in range(N_HB):
                nc.tensor.matmul(
                    psv,
                    lhsT=xT[:, hb, st * P:(st + 1) * P],
                    rhs=w_sb["v"][:, hb, :],
                    start=(hb == 0),
                    stop=(hb == N_HB - 1),
                )
            nc.vector.scalar_tensor_tensor(
                out=vp[:, st, :, 0:HD],
                in0=psv.rearrange("p (h d) -> p h d", h=HEADS_PER_CORE),
                scalar=1.0,
                in1=bvb.rearrange("p (h d) -> p h d", h=HEADS_PER_CORE),
                op0=mybir.AluOpType.mult,
                op1=mybir.AluOpType.add,
            )

        def phase1(ctx_ps0, pj0):
          # V' for tile st-1 is emitted after the transposes of tile st so
          # the PE never waits on the PSUM->SBUF eviction of its own tile.
          for st in range(N_ST):
            # stagger wq/wk staging (2 chunks per s-tile, after the early
            # x tiles + wv) so the x-tile prefetches keep their share of
            # HBM bandwidth
            if 3 <= st < 7:
                load_w("q", wq_d, (2 * (st - 3), 2 * (st - 3) + 1))
            elif 7 <= st < 11:
                load_w("k", wk_d, (2 * (st - 7), 2 * (st - 7) + 1))
            if st + 4 < N_ST:
                fetch_x(st + 4)
            xt = x_tiles[st]
            xb = xbp.tile([P, H], BF16, tag="xb", name=f"xb{st}")
            nc.scalar.copy(out=xb, in_=xt)
            for half in range(2):
                ps = ps_sc.tile([P, 4, P], BF16, tag="sc", name=f"xt{st}{half}")
                for q in range(4):
                    hb = half * 4 + q
                    nc.tensor.transpose(
                        ps[:, q, :], xb[:, hb * P:(hb + 1) * P], ident_b128)
                nc.vector.tensor_copy(
                    out=xT[:, half * 4:half * 4 + 4, st * P:(st + 1) * P],
                    in_=ps,
                )
            if st > 0:
                vprime(st - 1)
            if st >= 12:
                proj_chunk(0, st - 12, pool=ps_ctx, tag="ctx")
          vprime(N_ST - 1)

        # ---- attention for one head pair -------------------------------
        PD = 80  # 65 padded to a multiple of XBAR_TILE_SRC_ROWS (16)

        def new_ctx_ps(hp, qm):
            return [ps_ctx.tile([HD + 1, QM], FP32, tag="ctx",
                                name=f"ctx{hp}{qm}{hh}")
                    for hh in range(2)]

        # The Q/K projections for later head-pairs are paced globally at
        # ~7 matmuls per 8 iterations across ALL periods (instead of 1 per
        # iteration during hp0-2 and none during hp3): the exp instruction
        # only runs at its pure ~1005ns when the PE keeps the score feed a
        # full iteration ahead, and that needs per-iteration PE work at or
        # below the exp duration. Chunk deadlines (hp_next's qT/kT before
        # hp_next's periods) are met with large margin at this pacing.
        def make_proj_state():
            queue = []
            for nxt in (1, 2, 3):
                queue.append(("q", nxt, 0))
                queue.extend(("k", nxt, n) for n in range(N_QM))
                queue.extend(("q", nxt, n) for n in (1, 2, 3))
            return {"queue": queue, "ci": 0, "mi": 0, "tile": None}

        def attn_kc(hp, qm, kc, ctx_ps, pstate):
            # The last ctx of a period can only run after the last exp, so
            # the PE would starve through the period turnover. Hoisting the
            # first few chunks' proj+scores to the period start keeps the
            # PE (and therefore the ACT's score feed) saturated while the
            # ctx backlog drains through the exp-tile ring.
            hoist = (tc.high_priority(offset=6 * kc) if kc in (1, 2, 3, 4)
                     else _nullcontext())
            with hoist:
                if kc % 8 != 7 and pstate["ci"] < len(pstate["queue"]):
                    w_name, nxt, n = pstate["queue"][pstate["ci"]]
                    mi = pstate["mi"]
                    if mi == 0:
                        pstate["tile"] = ps_pj.tile(
                            [P, QM], FP32, tag="pj",
                            name=f"pj{w_name}{nxt}{n}")
                    nc.tensor.matmul(
                        pstate["tile"],
                        lhsT=w_sb[w_name][:, mi, nxt * P:(nxt + 1) * P],
                        rhs=xT[:, mi, n * QM:(n + 1) * QM],
                        start=(mi == 0),
                        stop=(mi == N_HB - 1),
                    )
                    if mi == N_HB - 1:
                        dst, bias = (qT, bqT) if w_name == "q" else (kT, bkT)
                        nc.vector.tensor_scalar_add(
                            out=dst[:, nxt, n * QM:(n + 1) * QM],
                            in0=pstate["tile"],
                            scalar1=bias[:, nxt:nxt + 1],
                        )
                        pstate["ci"] += 1
                        pstate["mi"] = 0
                    else:
                        pstate["mi"] = mi + 1
                sc = ps_sc.tile([P, 2, QM], FP32, tag="sc",
                                name=f"sc{hp}{qm}{kc}")
                for hh in range(2):
                    lo = hh * HD
                    nc.tensor.matmul(
                        sc[:, hh, :],
                        lhsT=kT[lo:lo + HD, hp, kc * P:(kc + 1) * P],
                        rhs=qT[lo:lo + HD, hp, qm * QM:(qm + 1) * QM],
                        start=True,
                        stop=True,
                        tile_position=(lo, 0),
                    )
            et = expp.tile([P, 2, QM], FP16, tag="exp")
            nc.scalar.activation(
                out=et, in_=sc,
                func=mybir.ActivationFunctionType.Exp,
                scale=0.125,
            )
            for hh in range(2):
                nc.tensor.matmul(
                    ctx_ps[hh],
                    lhsT=vp[:, kc, 2 * hp + hh, :],
                    rhs=et[:, hh, :],
                    start=(kc == 0),
                    stop=(kc == N_KC - 1),
                )

        def epilogue_a(ctx_ps, use_pe, ep_idx):
            """Drain the ctx accumulators out of PSUM and kick off the
            transposes. Emitted at the period boundary so the PSUM slots
            free quickly for the next period's accumulation."""
            tfulls = []
            for hh in range(2):
                csb = epil.tile([PD, QM], FP16, tag="ctxsb")
                if ep_idx < 2:
                    # rows 65:PD feed the xbar transpose as padding; each
                    # of the 4 ring buffers only needs zeroing once.
                    nc.vector.memset(csb[64:PD, :], 0.0)
                nc.vector.tensor_copy(out=csb[0:HD + 1, :], in_=ctx_ps[hh])
                if use_pe:
                    tp = ps_pj.tile([P, QM // P, HD + 2], FP16, tag="pj",
                                    name=f"tp{hh}")
                    for qs in range(QM // P):
                        nc.tensor.transpose(
                            tp[:, qs, 0:HD + 1],
                            csb[0:HD + 1, qs * P:(qs + 1) * P],
                            ident_b,
                        )
                    tfull = epil.tile([P, QM // P, HD + 1], FP16, tag="tpe")
                    nc.vector.tensor_copy(out=tfull, in_=tp[:, :, 0:HD + 1])
                else:
                    tfull = epil.tile([P, QM // P, PD], FP16, tag="tpsb")
                    for qs in range(QM // P):
                        nc.sync.dma_start_transpose(
                            out=tfull[:, qs, :],
                            in_=csb[:, qs * P:(qs + 1) * P],
                        )
                tfulls.append(tfull)
            return tfulls

        def epilogue_b(hp, qm, tfulls):
            """Reciprocal + scale + store. Deferred one period so these DVE
            ops are emitted after the next period's proj bias-adds and can
            never head-of-line-block them (the transposes they read were
            issued a full period earlier and are long done)."""
            for hh in range(2):
                tfull = tfulls[hh]
                rc = outp.tile([P, QM // P], FP32, tag="recip")
                nc.vector.reciprocal(out=rc, in_=tfull[:, :, HD:HD + 1])
                ot = outp.tile([P, QM // P, HD], FP32, tag="out")
                for qs in range(QM // P):
                    nc.vector.tensor_scalar_mul(
                        ot[:, qs, :], tfull[:, qs, 0:HD], rc[:, qs:qs + 1])
                row = qm * QM
                col = (2 * hp + hh) * HD
                nc.sync.dma_start(
                    out=out_d.ap()[row:row + QM, col:col + HD].rearrange(
                        "(a p) c -> p a c", p=P),
                    in_=ot,
                )

        phase1(None, None)
        pending = None
        pstate = make_proj_state()
        for hp in range(N_MT):
            for qm in range(N_QM):
                ctx_ps = new_ctx_ps(hp, qm)
                for kc in range(N_KC):
                    attn_kc(hp, qm, kc, ctx_ps, pstate)
                ep_idx = hp * N_QM + qm
                tfulls = epilogue_a(
                    ctx_ps,
                    use_pe=(hp == N_MT - 1 and qm == N_QM - 1),
                    ep_idx=ep_idx,
                )
                if pending is not None:
                    epilogue_b(*pending)
                pending = (hp, qm, tfulls)
        epilogue_b(*pending)


_NC_CACHE = None


def _get_nc():
    global _NC_CACHE
    if _NC_CACHE is None:
        _NC_CACHE = build()
    return _NC_CACHE


def make_in_maps(hidden_states, Wq, bq, Wk, bk, Wv, bv):
    hs = np.ascontiguousarray(np.asarray(hidden_states, dtype=np.float32))
    ws = {k: np.asarray(v, dtype=np.float32)
          for k, v in (("q", Wq), ("k", Wk), ("v", Wv))}
    bs = {k: np.asarray(v, dtype=np.float32)
          for k, v in (("q", bq), ("k", bk), ("v", bv))}
    in_maps = []
    for c in range(NCORES):
        b, g = c // 2, c % 2
        sl = slice(g * HG, (g + 1) * HG)
        in_maps.append({
            "x": np.ascontiguousarray(hs[b]),
            "wq": np.ascontiguousarray(ws["q"][:, sl]),
            "wk": np.ascontiguousarray(ws["k"][:, sl]),
            "wv": np.ascontiguousarray(ws["v"][:, sl]),
            "bq": np.ascontiguousarray(bs["q"][sl]),
            "bk": np.ascontiguousarray(bs["k"][sl]),
            "bv": np.ascontiguousarray(bs["v"][sl]),
        })
    return in_maps


def run(in_maps, trace=False):
    _ensure_profile_hook()
    nc = _get_nc()
    return run_bass_kernel_spmd(nc, in_maps, list(range(NCORES)), trace=trace)


def kernel(hidden_states, Wq, bq, Wk, bk, Wv, bv):
    in_maps = make_in_maps(hidden_states, Wq, bq, Wk, bk, Wv, bv)
    res = run(in_maps, trace=False)
    out = np.empty((B, S, H), dtype=np.float32)
    for c in range(NCORES):
        b, g = c // 2, c % 2
        out[b, :, g * HG:(g + 1) * HG] = res.results[c]["out"]
    return out


# revision 49
# speedup vs baseline: 1.2142x; 1.2142x over previous
"""BERT self-attention (B=4, S=2048, H=1024, 16 heads x 64) on 8 TRN2 NeuronCores.

Sharding: data-parallel over batch (4) x tensor-parallel over head-groups (2).
Core c handles batch c//2 and heads [8*(c%2), 8*(c%2)+8): it gets the full
hidden_states[b] plus the 512 W-columns/bias entries for its heads, and
produces out[b, :, 512*g : 512*(g+1)]. No cross-core communication.

Per-core kernel (bf16/fp16 matmuls, f32 accumulation in PSUM):
  xT   = transpose(x) via PE (bf16)               [1024h, 2048s]
  QT/KT = W.T @ xT  (+bias)                       [512hd, 2048s]
  V'   = xT.T @ Wv (+bias), 65 cols per head with an appended ones column
  per (head-pair, q-macro 512, k-chunk 128):
    scoresT[k, q] = KT_h[:, kc].T @ QT_h[:, qm]   (two heads row-packed, K=64)
    expT = exp(0.125 * scoresT)                   (ACT, fp16 out, N=1024/inst)
    ctxT[65, q] += V'_h[kc].T @ expT              (row 64 = softmax denominator)
    + one Q/K projection matmul of the NEXT head-pair (fills the PE gap
      while ACT paces the loop at ~1.1us/k-chunk)
  epilogue: ctxT -> hardware DMA-transpose -> [q, 65]; multiply by
  reciprocal denominator (GPSIMD); batched DMA out via HWDGE.

Schedule notes (from perfetto traces):
 - ACT exp (FD=1024 from PSUM) is the loop pacer at ~1.07-1.11us/iter;
   the PE instruction stream (ctx pair + row-packed score pair + one
   proj MM) is ~1.08us/iter, so both engines run ~100% in steady state.
   PSUM's 8 banks are exactly spent (2x2 score double-buffer + 2 ctx
   accumulators + 2 proj accumulators), which caps the exp FD at 1024.
 - The epilogue is split: the PSUM-draining copies + DMA-transposes are
   issued at the period boundary, but the reciprocal/scale/store half is
   deferred one full period so its DVE ops sit after the next period's
   proj bias-adds in the DVE FIFO and can never head-of-line-block them
   (that stall idled ACT ~1.7us per boundary and HAM-re-throttled PE).
 - The last ctx of a period can only run after the last exp, which
   starves the PE through the turnover (the 2-slot score ring can't run
   ahead). Chunks kc=1..4's proj+scores are priority-hoisted to the
   period start so the boundary window stays full; the ctx backlog
   drains through the 10-deep exp-tile ring. Hoisting more chunks
   back-fires (priority ties scramble the score-ring order).
 - All staging DMAs go through the sync-engine HWDGE rings (SWDGE
   DIRECT2D enqueues cost ~640ns each on the GPSIMD engine), with x
   prefetched ahead of the wq/wk staging (DMA rings round-robin, so
   enqueue order decides arrival order under bandwidth contention).
 - The Q/K projection interleave is paced globally at 7 matmuls per 8
   iterations across all 16 periods (not 1/iter during hp0-2 only): the
   exp only hits its pure ~1005ns when per-iteration PE work stays at or
   below it, and chunk deadlines hold with margin at this pacing.
Measured on TRN2: ~368us HW exec (baseline 412us), rel l2 err ~3.7e-3.
"""

import sys
import types

sys.path.insert(0, "/opt/trn_rl_repo")

import numpy as np

import concourse.bass as bass
import concourse.tile as tile
from concourse import bacc, mybir
from concourse.bass_utils import run_bass_kernel_spmd
from concourse.masks import make_identity

B, S, H = 4, 2048, 1024
NH, HD = 16, 64
NCORES = 8
HEADS_PER_CORE = NH // 2      # 8 heads per core
HG = HEADS_PER_CORE * HD      # 512 = per-core head width
P = 128
QM = 512                      # q macro-tile
N_QM = S // QM                # 4
N_KC = S // P                 # 16 k chunks
N_ST = S // P                 # 16 s tiles
N_HB = H // P                 # 8 h chunks (contraction)
N_MT = HG // P                # 4 hd m-tiles

FP32 = mybir.dt.float32
BF16 = mybir.dt.bfloat16
FP16 = mybir.dt.float16


def _ensure_profile_hook():
    """The image's antenv lacks axon_hooks; shim it so trace=True works."""
    try:
        from antenv.axon_hooks import get_axon_ntff_profile_hook  # noqa: F401
        return
    except ImportError:
        pass
    try:
        from trn_agent_boot.trn_boot import _ntff_profile_via_ctypes
    except ImportError:
        return
    hook = _ntff_profile_via_ctypes("/opt/axon/libaxon_pjrt.so")
    mod = types.ModuleType("antenv.axon_hooks")
    mod.get_axon_ntff_profile_hook = lambda: hook
    mod.set_axon_ntff_profile_hook = lambda h: None
    sys.modules["antenv.axon_hooks"] = mod


def build():
    nc = bacc.Bacc("TRN2", target_bir_lowering=False, debug=False,
                   num_devices=NCORES)

    x_d = nc.declare_dram_parameter("x", [S, H], FP32, isOutput=False)
    wq_d = nc.declare_dram_parameter("wq", [H, HG], FP32, isOutput=False)
    wk_d = nc.declare_dram_parameter("wk", [H, HG], FP32, isOutput=False)
    wv_d = nc.declare_dram_parameter("wv", [H, HG], FP32, isOutput=False)
    bq_d = nc.declare_dram_parameter("bq", [HG], FP32, isOutput=False)
    bk_d = nc.declare_dram_parameter("bk", [HG], FP32, isOutput=False)
    bv_d = nc.declare_dram_parameter("bv", [HG], FP32, isOutput=False)
    out_d = nc.declare_dram_parameter("out", [S, HG], FP32, isOutput=True)

    with tile.TileContext(nc) as tc:
        _build_body(nc, tc, x_d, (wq_d, wk_d, wv_d), (bq_d, bk_d, bv_d), out_d)

    nc.finalize()
    return nc


def _build_body(nc, tc, x_d, w_d, b_d, out_d):
    wq_d, wk_d, wv_d = w_d
    bq_d, bk_d, bv_d = b_d

    import contextlib
    from contextlib import nullcontext as _nullcontext
    ctx = contextlib.ExitStack()
    with ctx:
        const = ctx.enter_context(tc.tile_pool(name="const", bufs=1))
        xf = ctx.enter_context(tc.tile_pool(name="xf", bufs=4))
        xbp = ctx.enter_context(tc.tile_pool(name="xbp", bufs=3))
        big = ctx.enter_context(tc.tile_pool(name="big", bufs=1))
        wstage = ctx.enter_context(tc.tile_pool(name="wstage", bufs=3))
        expp = ctx.enter_context(tc.tile_pool(name="expp", bufs=10))
        epil = ctx.enter_context(tc.tile_pool(name="epil", bufs=4))
        outp = ctx.enter_context(tc.tile_pool(name="outp", bufs=4))
        # PSUM budget (8 banks): ps_sc = 2 x 2-bank slots (scores double
        # buffer), ps_ctx = 2 x 1-bank slots (ctx accumulators), ps_pj =
        # 2 x 1-bank slots (V'/QK projection accumulators).
        ps_sc = ctx.enter_context(
            tc.tile_pool(name="ps_sc", bufs=2, space="PSUM"))
        ps_ctx = ctx.enter_context(
            tc.tile_pool(name="ps_ctx", bufs=2, space="PSUM"))
        ps_pj = ctx.enter_context(
            tc.tile_pool(name="ps_pj", bufs=2, space="PSUM"))

        # ---- x prefetch first so the first s-tiles land ASAP -----------
        x_tiles = {}

        def fetch_x(st):
            xt = xf.tile([P, H], FP32, tag="x", name=f"x{st}")
            nc.sync.dma_start(out=xt, in_=x_d.ap()[st * P:(st + 1) * P, :])
            x_tiles[st] = xt

        # ---- weights: staged via sync HWDGE, cast on DVE ---------------
        w_sb = {}
        for name, wd in (("q", wq_d), ("k", wk_d), ("v", wv_d)):
            w_sb[name] = big.tile([P, N_HB, HG], BF16, tag=f"w{name}",
                                  name=f"w{name}")

        def load_w(name, wd, ks):
            for k in ks:
                stg = wstage.tile([P, HG], FP32, tag="wstg", name=f"w{name}{k}")
                nc.sync.dma_start(out=stg, in_=wd.ap()[k * P:(k + 1) * P, :])
                nc.vector.tensor_copy(out=w_sb[name][:, k, :], in_=stg)

        # interleave the wv staging with the x prefetches: the DMA rings
        # round-robin, so a solid 2MB wv burst would delay x1-x3 (whose
        # casts/transposes gate the whole phase1 pipeline)
        fetch_x(0)
        fetch_x(1)
        load_w("v", wv_d, (0, 1))
        fetch_x(2)
        load_w("v", wv_d, (2, 3))
        fetch_x(3)
        load_w("v", wv_d, (4, 5))

        # ---- constants -------------------------------------------------
        ident_b128 = const.tile([P, P], BF16)
        make_identity(nc, ident_b128)
        ident_h = const.tile([HD + 1, HD + 1], FP16)
        make_identity(nc, ident_h)
        ident_b = ident_h
        bqT = const.tile([P, N_MT], FP32)
        nc.sync.dma_start(out=bqT, in_=bq_d.ap().rearrange("(o p) -> p o", p=P))
        bkT = const.tile([P, N_MT], FP32)
        nc.sync.dma_start(out=bkT, in_=bk_d.ap().rearrange("(o p) -> p o", p=P))
        bv_ap = bv_d.ap()
        bvb = const.tile([P, HG], FP32)
        nc.sync.dma_start(
            out=bvb,
            in_=bass.AP(tensor=bv_ap.tensor, offset=bv_ap.offset,
                        ap=[[0, P]] + [list(a) for a in bv_ap.ap]),
        )

        load_w("v", wv_d, (6, 7))

        # ---- per s-tile: load x, transpose to xT, project V' -----------
        xT = big.tile([P, N_HB, S], BF16, tag="xT")
        vp = big.tile([P, N_ST, HEADS_PER_CORE, HD + 1], BF16, tag="vp")
        # only the appended ones-column needs initializing; cols 0:HD are
        # fully written by the V' bias add below
        nc.vector.memset(vp[:, :, :, HD:HD + 1], 1.0)

        qT = big.tile([P, N_MT, S], BF16, tag="qT")
        kT = big.tile([P, N_MT, S], BF16, tag="kT")

        def proj_chunk(mt, n, pool=None, tag=None):
            pool = pool or ps_pj
            tag = tag or "pj"
            for w_name, dst, bias in (("q", qT, bqT), ("k", kT, bkT)):
                ps = pool.tile([P, QM], FP32, tag=tag,
                               name=f"proj{w_name}{mt}{n}")
                for k in range(N_HB):
                    nc.tensor.matmul(
                        ps,
                        lhsT=w_sb[w_name][:, k, mt * P:(mt + 1) * P],
                        rhs=xT[:, k, n * QM:(n + 1) * QM],
                        start=(k == 0),
                        stop=(k == N_HB - 1),
                    )
                nc.vector.tensor_scalar_add(
                    out=dst[:, mt, n * QM:(n + 1) * QM],
                    in0=ps,
                    scalar1=bias[:, mt:mt + 1],
                )

        def vprime(st):
            psv = ps_pj.tile([P, HG], FP32, tag="pj", name=f"v{st}")
            for hb in range(N_HB):
                nc.tensor.matmul(
                    psv,
                    lhsT=xT[:, hb, st * P:(st + 1) * P],
                    rhs=w_sb["v"][:, hb, :],
                    start=(hb == 0),
                    stop=(hb == N_HB - 1),
                )
            nc.vector.scalar_tensor_tensor(
                out=vp[:, st, :, 0:HD],
                in0=psv.rearrange("p (h d) -> p h d", h=HEADS_PER_CORE),
                scalar=1.0,
                in1=bvb.rearrange("p (h d) -> p h d", h=HEADS_PER_CORE),
                op0=mybir.AluOpType.mult,
                op1=mybir.AluOpType.add,
            )

        def phase1(ctx_ps0, pj0):
          # V' for tile st-1 is emitted after the transposes of tile st so
          # the PE never waits on the PSUM->SBUF eviction of its own tile.
          for st in range(N_ST):
            # stagger wq/wk staging (2 chunks per s-tile, after the early
            # x tiles + wv) so the x-tile prefetches keep their share of
            # HBM bandwidth
            if 3 <= st < 7:
                load_w("q", wq_d, (2 * (st - 3), 2 * (st - 3) + 1))
            elif 7 <= st < 11:
                load_w("k", wk_d, (2 * (st - 7), 2 * (st - 7) + 1))
            if st + 4 < N_ST:
                fetch_x(st + 4)
            xt = x_tiles[st]
            xb = xbp.tile([P, H], BF16, tag="xb", name=f"xb{st}")
            nc.scalar.copy(out=xb, in_=xt)
            for half in range(2):
                ps = ps_sc.tile([P, 4, P], BF16, tag="sc", name=f"xt{st}{half}")
                for q in range(4):
                    hb = half * 4 + q
                    nc.tensor.transpose(
                        ps[:, q, :], xb[:, hb * P:(hb + 1) * P], ident_b128)
                nc.vector.tensor_copy(
                    out=xT[:, half * 4:half * 4 + 4, st * P:(st + 1) * P],
                    in_=ps,
                )
            if st > 0:
                vprime(st - 1)
            if st >= 12:
                proj_chunk(0, st - 12, pool=ps_ctx, tag="ctx")
          vprime(N_ST - 1)

        # ---- attention for one head pair -------------------------------
        PD = 80  # 65 padded to a multiple of XBAR_TILE_SRC_ROWS (16)

        def new_ctx_ps(hp, qm):
            return [ps_ctx.tile([HD + 1, QM], FP32, tag="ctx",
                                name=f"ctx{hp}{qm}{hh}")
                    for hh in range(2)]

        # The Q/K projections for later head-pairs are paced globally at
        # ~7 matmuls per 8 iterations across ALL periods (instead of 1 per
        # iteration during hp0-2 and none during hp3): the exp instruction
        # only runs at its pure ~1005ns when the PE keeps the score feed a
        # full iteration ahead, and that needs per-iteration PE work at or
        # below the exp duration. Chunk deadlines (hp_next's qT/kT before
        # hp_next's periods) are met with large margin at this pacing.
        def make_proj_state():
            queue = []
            for nxt in (1, 2, 3):
                queue.append(("q", nxt, 0))
                queue.extend(("k", nxt, n) for n in range(N_QM))
                queue.extend(("q", nxt, n) for n in (1, 2, 3))
            return {"queue": queue, "ci": 0, "mi": 0, "tile": None}

        def attn_kc(hp, qm, kc, ctx_ps, pstate):
            # The last ctx of a period can only run after the last exp, so
            # the PE would starve through the period turnover. Hoisting the
            # first few chunks' proj+scores to the period start keeps the
            # PE (and therefore the ACT's score feed) saturated while the
            # ctx backlog drains through the exp-tile ring.
            hoist = (tc.high_priority(offset=6 * kc) if kc in (1, 2, 3, 4)
                     else _nullcontext())
            with hoist:
                if kc % 8 != 7 and pstate["ci"] < len(pstate["queue"]):
                    w_name, nxt, n = pstate["queue"][pstate["ci"]]
                    mi = pstate["mi"]
                    if mi == 0:
                        pstate["tile"] = ps_pj.tile(
                            [P, QM], FP32, tag="pj",
                            name=f"pj{w_name}{nxt}{n}")
                    nc.tensor.matmul(
                        pstate["tile"],
                        lhsT=w_sb[w_name][:, mi, nxt * P:(nxt + 1) * P],
                        rhs=xT[:, mi, n * QM:(n + 1) * QM],
                        start=(mi == 0),
                        stop=(mi == N_HB - 1),
                    )
                    if mi == N_HB - 1:
                        dst, bias = (qT, bqT) if w_name == "q" else (kT, bkT)
                        nc.vector.tensor_scalar_add(
                            out=dst[:, nxt, n * QM:(n + 1) * QM],
                            in0=pstate["tile"],
                            scalar1=bias[:, nxt:nxt + 1],
                        )
                        pstate["ci"] += 1
                        pstate["mi"] = 0
                    else:
                        pstate["mi"] = mi + 1
                sc = ps_sc.tile([P, 2, QM], FP32, tag="sc",
                                name=f"sc{hp}{qm}{kc}")
                for hh in range(2):
                    lo = hh * HD
                    nc.tensor.matmul(
                        sc[:, hh, :],
                        lhsT=kT[lo:lo + HD, hp, kc * P:(kc + 1) * P],
                        rhs=qT[lo:lo + HD, hp, qm * QM:(qm + 1) * QM],
                        start=True,
                        stop=True,
                        tile_position=(lo, 0),
                    )
            et = expp.tile([P, 2, QM], FP16, tag="exp")
            nc.scalar.activation(
                out=et, in_=sc,
                func=mybir.ActivationFunctionType.Exp,
                scale=0.125,
            )
            for hh in range(2):
                nc.tensor.matmul(
                    ctx_ps[hh],
                    lhsT=vp[:, kc, 2 * hp + hh, :],
                    rhs=et[:, hh, :],
                    start=(kc == 0),
                    stop=(kc == N_KC - 1),
                )

        def epilogue_a(ctx_ps, use_pe, ep_idx):
            """Drain the ctx accumulators out of PSUM and kick off the
            transposes. Emitted at the period boundary so the PSUM slots
            free quickly for the next period's accumulation."""
            tfulls = []
            for hh in range(2):
                csb = epil.tile([PD, QM], FP16, tag="ctxsb")
                if ep_idx < 2:
                    # rows 65:PD feed the xbar transpose as padding; each
                    # of the 4 ring buffers only needs zeroing once.
                    nc.vector.memset(csb[64:PD, :], 0.0)
                nc.vector.tensor_copy(out=csb[0:HD + 1, :], in_=ctx_ps[hh])
                if use_pe:
                    tp = ps_pj.tile([P, QM // P, HD + 2], FP16, tag="pj",
                                    name=f"tp{hh}")
                    for qs in range(QM // P):
                        nc.tensor.transpose(
                            tp[:, qs, 0:HD + 1],
                            csb[0:HD + 1, qs * P:(qs + 1) * P],
                            ident_b,
                        )
                    tfull = epil.tile([P, QM // P, HD + 1], FP16, tag="tpe")
                    nc.vector.tensor_copy(out=tfull, in_=tp[:, :, 0:HD + 1])
                else:
                    tfull = epil.tile([P, QM // P, PD], FP16, tag="tpsb")
                    for qs in range(QM // P):
                        nc.sync.dma_start_transpose(
                            out=tfull[:, qs, :],
                            in_=csb[:, qs * P:(qs + 1) * P],
                        )
                tfulls.append(tfull)
            return tfulls

        def epilogue_b(hp, qm, tfulls):
            """Reciprocal + scale + store. Deferred one period so these DVE
            ops are emitted after the next period's proj bias-adds and can
            never head-of-line-block them (the transposes they read were
            issued a full period earlier and are long done)."""
            for hh in range(2):
                tfull = tfulls[hh]
                rc = outp.tile([P, QM // P], FP32, tag="recip")
                nc.vector.reciprocal(out=rc, in_=tfull[:, :, HD:HD + 1])
                ot = outp.tile([P, QM // P, HD], FP32, tag="out")
                for qs in range(QM // P):
                    nc.vector.tensor_scalar_mul(
                        ot[:, qs, :], tfull[:, qs, 0:HD], rc[:, qs:qs + 1])
                row = qm * QM
                col = (2 * hp + hh) * HD
                nc.sync.dma_start(
                    out=out_d.ap()[row:row + QM, col:col + HD].rearrange(
                        "(a p) c -> p a c", p=P),
                    in_=ot,
                )

        phase1(None, None)
        pending = None
        pstate = make_proj_state()
        for hp in range(N_MT):
            for qm in range(N_QM):
                ctx_ps = new_ctx_ps(hp, qm)
                for kc in range(N_KC):
                    attn_kc(hp, qm, kc, ctx_ps, pstate)
                ep_idx = hp * N_QM + qm
                tfulls = epilogue_a(
                    ctx_ps,
                    use_pe=(hp == N_MT - 1 and qm == N_QM - 1),
                    ep_idx=ep_idx,
                )
                if pending is not None:
                    epilogue_b(*pending)
                pending = (hp, qm, tfulls)
        epilogue_b(*pending)


_NC_CACHE = None


def _get_nc():
    global _NC_CACHE
    if _NC_CACHE is None:
        _NC_CACHE = build()
    return _NC_CACHE


def make_in_maps(hidden_states, Wq, bq, Wk, bk, Wv, bv):
    hs = np.ascontiguousarray(np.asarray(hidden_states, dtype=np.float32))
    ws = {k: np.asarray(v, dtype=np.float32)
          for k, v in (("q", Wq), ("k", Wk), ("v", Wv))}
    bs = {k: np.asarray(v, dtype=np.float32)
          for k, v in (("q", bq), ("k", bk), ("v", bv))}
    in_maps = []
    for c in range(NCORES):
        b, g = c // 2, c % 2
        sl = slice(g * HG, (g + 1) * HG)
        in_maps.append({
            "x": np.ascontiguousarray(hs[b]),
            "wq": np.ascontiguousarray(ws["q"][:, sl]),
            "wk": np.ascontiguousarray(ws["k"][:, sl]),
            "wv": np.ascontiguousarray(ws["v"][:, sl]),
            "bq": np.ascontiguousarray(bs["q"][sl]),
            "bk": np.ascontiguousarray(bs["k"][sl]),
            "bv": np.ascontiguousarray(bs["v"][sl]),
        })
    return in_maps


def run(in_maps, trace=False):
    _ensure_profile_hook()
    nc = _get_nc()
    return run_bass_kernel_spmd(nc, in_maps, list(range(NCORES)), trace=trace)


def kernel(hidden_states, Wq, bq, Wk, bk, Wv, bv):
    in_maps = make_in_maps(hidden_states, Wq, bq, Wk, bk, Wv, bv)
    res = run(in_maps, trace=False)
    out = np.empty((B, S, H), dtype=np.float32)
    for c in range(NCORES):
        b, g = c // 2, c % 2
        out[b, :, g * HG:(g + 1) * HG] = res.results[c]["out"]
    return out


# revision 52
# speedup vs baseline: 1.2159x; 1.0014x over previous
"""BERT self-attention (B=4, S=2048, H=1024, 16 heads x 64) on 8 TRN2 NeuronCores.

Sharding: data-parallel over batch (4) x tensor-parallel over head-groups (2).
Core c handles batch c//2 and heads [8*(c%2), 8*(c%2)+8): it gets the full
hidden_states[b] plus the 512 W-columns/bias entries for its heads, and
produces out[b, :, 512*g : 512*(g+1)]. No cross-core communication.

Per-core kernel (bf16/fp16 matmuls, f32 accumulation in PSUM):
  xT   = transpose(x) via PE (bf16)               [1024h, 2048s]
  QT/KT = W.T @ xT  (+bias)                       [512hd, 2048s]
  V'   = xT.T @ Wv (+bias), 65 cols per head with an appended ones column
  per (head-pair, q-macro 512, k-chunk 128):
    scoresT[k, q] = KT_h[:, kc].T @ QT_h[:, qm]   (two heads row-packed, K=64)
    expT = exp(0.125 * scoresT)                   (ACT, fp16 out, N=1024/inst)
    ctxT[65, q] += V'_h[kc].T @ expT              (row 64 = softmax denominator)
    + one Q/K projection matmul of the NEXT head-pair (fills the PE gap
      while ACT paces the loop at ~1.1us/k-chunk)
  epilogue: ctxT -> hardware DMA-transpose -> [q, 65]; multiply by
  reciprocal denominator (GPSIMD); batched DMA out via HWDGE.

Schedule notes (from perfetto traces):
 - ACT exp (FD=1024 from PSUM) is the loop pacer at ~1.07-1.11us/iter;
   the PE instruction stream (ctx pair + row-packed score pair + one
   proj MM) is ~1.08us/iter, so both engines run ~100% in steady state.
   PSUM's 8 banks are exactly spent (2x2 score double-buffer + 2 ctx
   accumulators + 2 proj accumulators), which caps the exp FD at 1024.
 - The epilogue is split: the PSUM-draining copies + DMA-transposes are
   issued at the period boundary, but the reciprocal/scale/store half is
   deferred one full period so its DVE ops sit after the next period's
   proj bias-adds in the DVE FIFO and can never head-of-line-block them
   (that stall idled ACT ~1.7us per boundary and HAM-re-throttled PE).
 - The last ctx of a period can only run after the last exp, which
   starves the PE through the turnover (the 2-slot score ring can't run
   ahead). Chunks kc=1..4's proj+scores are priority-hoisted to the
   period start so the boundary window stays full; the ctx backlog
   drains through the 10-deep exp-tile ring. Hoisting more chunks
   back-fires (priority ties scramble the score-ring order).
 - All staging DMAs go through the sync-engine HWDGE rings (SWDGE
   DIRECT2D enqueues cost ~640ns each on the GPSIMD engine), with x
   prefetched ahead of the wq/wk staging (DMA rings round-robin, so
   enqueue order decides arrival order under bandwidth contention).
 - The Q/K projection interleave is paced globally at 7 matmuls per 8
   iterations across all 16 periods (not 1/iter during hp0-2 only): the
   exp only hits its pure ~1005ns when per-iteration PE work stays at or
   below it, and chunk deadlines hold with margin at this pacing.
Measured on TRN2: ~368us HW exec (baseline 412us), rel l2 err ~3.7e-3.
"""

import sys
import types

sys.path.insert(0, "/opt/trn_rl_repo")

import numpy as np

import concourse.bass as bass
import concourse.tile as tile
from concourse import bacc, mybir
from concourse.bass_utils import run_bass_kernel_spmd
from concourse.masks import make_identity

B, S, H = 4, 2048, 1024
NH, HD = 16, 64
NCORES = 8
HEADS_PER_CORE = NH // 2      # 8 heads per core
HG = HEADS_PER_CORE * HD      # 512 = per-core head width
P = 128
QM = 512                      # q macro-tile
N_QM = S // QM                # 4
N_KC = S // P                 # 16 k chunks
N_ST = S // P                 # 16 s tiles
N_HB = H // P                 # 8 h chunks (contraction)
N_MT = HG // P                # 4 hd m-tiles

FP32 = mybir.dt.float32
BF16 = mybir.dt.bfloat16
FP16 = mybir.dt.float16


def _ensure_profile_hook():
    """The image's antenv lacks axon_hooks; shim it so trace=True works."""
    try:
        from antenv.axon_hooks import get_axon_ntff_profile_hook  # noqa: F401
        return
    except ImportError:
        pass
    try:
        from trn_agent_boot.trn_boot import _ntff_profile_via_ctypes
    except ImportError:
        return
    hook = _ntff_profile_via_ctypes("/opt/axon/libaxon_pjrt.so")
    mod = types.ModuleType("antenv.axon_hooks")
    mod.get_axon_ntff_profile_hook = lambda: hook
    mod.set_axon_ntff_profile_hook = lambda h: None
    sys.modules["antenv.axon_hooks"] = mod


def build():
    nc = bacc.Bacc("TRN2", target_bir_lowering=False, debug=False,
                   num_devices=NCORES)

    x_d = nc.declare_dram_parameter("x", [S, H], FP32, isOutput=False)
    wq_d = nc.declare_dram_parameter("wq", [H, HG], FP32, isOutput=False)
    wk_d = nc.declare_dram_parameter("wk", [H, HG], FP32, isOutput=False)
    wv_d = nc.declare_dram_parameter("wv", [H, HG], FP32, isOutput=False)
    bq_d = nc.declare_dram_parameter("bq", [HG], FP32, isOutput=False)
    bk_d = nc.declare_dram_parameter("bk", [HG], FP32, isOutput=False)
    bv_d = nc.declare_dram_parameter("bv", [HG], FP32, isOutput=False)
    out_d = nc.declare_dram_parameter("out", [S, HG], FP32, isOutput=True)

    with tile.TileContext(nc) as tc:
        _build_body(nc, tc, x_d, (wq_d, wk_d, wv_d), (bq_d, bk_d, bv_d), out_d)

    nc.finalize()
    return nc


def _build_body(nc, tc, x_d, w_d, b_d, out_d):
    wq_d, wk_d, wv_d = w_d
    bq_d, bk_d, bv_d = b_d

    import contextlib
    from contextlib import nullcontext as _nullcontext
    ctx = contextlib.ExitStack()
    with ctx:
        const = ctx.enter_context(tc.tile_pool(name="const", bufs=1))
        xf = ctx.enter_context(tc.tile_pool(name="xf", bufs=4))
        xbp = ctx.enter_context(tc.tile_pool(name="xbp", bufs=3))
        big = ctx.enter_context(tc.tile_pool(name="big", bufs=1))
        wstage = ctx.enter_context(tc.tile_pool(name="wstage", bufs=3))
        expp = ctx.enter_context(tc.tile_pool(name="expp", bufs=10))
        epil = ctx.enter_context(tc.tile_pool(name="epil", bufs=4))
        outp = ctx.enter_context(tc.tile_pool(name="outp", bufs=4))
        # PSUM budget (8 banks): ps_sc = 2 x 2-bank slots (scores double
        # buffer), ps_ctx = 2 x 1-bank slots (ctx accumulators), ps_pj =
        # 2 x 1-bank slots (V'/QK projection accumulators).
        ps_sc = ctx.enter_context(
            tc.tile_pool(name="ps_sc", bufs=2, space="PSUM"))
        ps_ctx = ctx.enter_context(
            tc.tile_pool(name="ps_ctx", bufs=2, space="PSUM"))
        ps_pj = ctx.enter_context(
            tc.tile_pool(name="ps_pj", bufs=2, space="PSUM"))

        # ---- x prefetch first so the first s-tiles land ASAP -----------
        x_tiles = {}

        def fetch_x(st):
            xt = xf.tile([P, H], FP32, tag="x", name=f"x{st}")
            nc.sync.dma_start(out=xt, in_=x_d.ap()[st * P:(st + 1) * P, :])
            x_tiles[st] = xt

        for st in range(4):
            fetch_x(st)

        # ---- constants -------------------------------------------------
        ident_b128 = const.tile([P, P], BF16)
        make_identity(nc, ident_b128)
        ident_h = const.tile([HD + 1, HD + 1], FP16)
        make_identity(nc, ident_h)
        ident_b = ident_h
        bqT = const.tile([P, N_MT], FP32)
        nc.sync.dma_start(out=bqT, in_=bq_d.ap().rearrange("(o p) -> p o", p=P))
        bkT = const.tile([P, N_MT], FP32)
        nc.sync.dma_start(out=bkT, in_=bk_d.ap().rearrange("(o p) -> p o", p=P))
        bv_ap = bv_d.ap()
        bvb = const.tile([P, HG], FP32)
        nc.sync.dma_start(
            out=bvb,
            in_=bass.AP(tensor=bv_ap.tensor, offset=bv_ap.offset,
                        ap=[[0, P]] + [list(a) for a in bv_ap.ap]),
        )

        # ---- weights: staged via sync HWDGE, cast on DVE ---------------
        w_sb = {}
        for name, wd in (("q", wq_d), ("k", wk_d), ("v", wv_d)):
            w_sb[name] = big.tile([P, N_HB, HG], BF16, tag=f"w{name}",
                                  name=f"w{name}")

        def load_w(name, wd, ks):
            for k in ks:
                stg = wstage.tile([P, HG], FP32, tag="wstg", name=f"w{name}{k}")
                nc.sync.dma_start(out=stg, in_=wd.ap()[k * P:(k + 1) * P, :])
                nc.vector.tensor_copy(out=w_sb[name][:, k, :], in_=stg)

        load_w("v", wv_d, range(N_HB))

        # ---- per s-tile: load x, transpose to xT, project V' -----------
        xT = big.tile([P, N_HB, S], BF16, tag="xT")
        vp = big.tile([P, N_ST, HEADS_PER_CORE, HD + 1], BF16, tag="vp")
        # only the appended ones-column needs initializing; cols 0:HD are
        # fully written by the V' bias add below
        nc.vector.memset(vp[:, :, :, HD:HD + 1], 1.0)

        qT = big.tile([P, N_MT, S], BF16, tag="qT")
        kT = big.tile([P, N_MT, S], BF16, tag="kT")

        def proj_chunk(mt, n, pool=None, tag=None):
            pool = pool or ps_pj
            tag = tag or "pj"
            for w_name, dst, bias in (("q", qT, bqT), ("k", kT, bkT)):
                ps = pool.tile([P, QM], FP32, tag=tag,
                               name=f"proj{w_name}{mt}{n}")
                for k in range(N_HB):
                    nc.tensor.matmul(
                        ps,
                        lhsT=w_sb[w_name][:, k, mt * P:(mt + 1) * P],
                        rhs=xT[:, k, n * QM:(n + 1) * QM],
                        start=(k == 0),
                        stop=(k == N_HB - 1),
                    )
                nc.vector.tensor_scalar_add(
                    out=dst[:, mt, n * QM:(n + 1) * QM],
                    in0=ps,
                    scalar1=bias[:, mt:mt + 1],
                )

        def vprime(st):
            psv = ps_pj.tile([P, HG], FP32, tag="pj", name=f"v{st}")
            for hb in range(N_HB):
                nc.tensor.matmul(
                    psv,
                    lhsT=xT[:, hb, st * P:(st + 1) * P],
                    rhs=w_sb["v"][:, hb, :],
                    start=(hb == 0),
                    stop=(hb == N_HB - 1),
                )
            nc.vector.scalar_tensor_tensor(
                out=vp[:, st, :, 0:HD],
                in0=psv.rearrange("p (h d) -> p h d", h=HEADS_PER_CORE),
                scalar=1.0,
                in1=bvb.rearrange("p (h d) -> p h d", h=HEADS_PER_CORE),
                op0=mybir.AluOpType.mult,
                op1=mybir.AluOpType.add,
            )

        def phase1(ctx_ps0, pj0):
          # V' for tile st-1 is emitted after the transposes of tile st so
          # the PE never waits on the PSUM->SBUF eviction of its own tile.
          for st in range(N_ST):
            # stagger wq/wk staging (2 chunks per s-tile, after the early
            # x tiles + wv) so the x-tile prefetches keep their share of
            # HBM bandwidth
            if 3 <= st < 7:
                load_w("q", wq_d, (2 * (st - 3), 2 * (st - 3) + 1))
            elif 7 <= st < 11:
                load_w("k", wk_d, (2 * (st - 7), 2 * (st - 7) + 1))
            if st + 4 < N_ST:
                fetch_x(st + 4)
            xt = x_tiles[st]
            xb = xbp.tile([P, H], BF16, tag="xb", name=f"xb{st}")
            nc.scalar.copy(out=xb, in_=xt)
            for half in range(2):
                ps = ps_sc.tile([P, 4, P], BF16, tag="sc", name=f"xt{st}{half}")
                for q in range(4):
                    hb = half * 4 + q
                    nc.tensor.transpose(
                        ps[:, q, :], xb[:, hb * P:(hb + 1) * P], ident_b128)
                nc.vector.tensor_copy(
                    out=xT[:, half * 4:half * 4 + 4, st * P:(st + 1) * P],
                    in_=ps,
                )
            if st > 0:
                vprime(st - 1)
            if st >= 12:
                proj_chunk(0, st - 12, pool=ps_ctx, tag="ctx")
          vprime(N_ST - 1)

        # ---- attention for one head pair -------------------------------
        PD = 80  # 65 padded to a multiple of XBAR_TILE_SRC_ROWS (16)

        def new_ctx_ps(hp, qm):
            return [ps_ctx.tile([HD + 1, QM], FP32, tag="ctx",
                                name=f"ctx{hp}{qm}{hh}")
                    for hh in range(2)]

        # The Q/K projections for later head-pairs are paced globally at
        # ~7 matmuls per 8 iterations across ALL periods (instead of 1 per
        # iteration during hp0-2 and none during hp3): the exp instruction
        # only runs at its pure ~1005ns when the PE keeps the score feed a
        # full iteration ahead, and that needs per-iteration PE work at or
        # below the exp duration. Chunk deadlines (hp_next's qT/kT before
        # hp_next's periods) are met with large margin at this pacing.
        def make_proj_state():
            queue = []
            for nxt in (1, 2, 3):
                queue.append(("q", nxt, 0))
                queue.extend(("k", nxt, n) for n in range(N_QM))
                queue.extend(("q", nxt, n) for n in (1, 2, 3))
            return {"queue": queue, "ci": 0, "mi": 0, "tile": None}

        def attn_kc(hp, qm, kc, ctx_ps, pstate):
            # The last ctx of a period can only run after the last exp, so
            # the PE would starve through the period turnover. Hoisting the
            # first few chunks' proj+scores to the period start keeps the
            # PE (and therefore the ACT's score feed) saturated while the
            # ctx backlog drains through the exp-tile ring.
            hoist = (tc.high_priority(offset=6 * kc) if kc in (1, 2, 3, 4)
                     else _nullcontext())
            with hoist:
                if kc % 8 != 7 and pstate["ci"] < len(pstate["queue"]):
                    w_name, nxt, n = pstate["queue"][pstate["ci"]]
                    mi = pstate["mi"]
                    if mi == 0:
                        pstate["tile"] = ps_pj.tile(
                            [P, QM], FP32, tag="pj",
                            name=f"pj{w_name}{nxt}{n}")
                    nc.tensor.matmul(
                        pstate["tile"],
                        lhsT=w_sb[w_name][:, mi, nxt * P:(nxt + 1) * P],
                        rhs=xT[:, mi, n * QM:(n + 1) * QM],
                        start=(mi == 0),
                        stop=(mi == N_HB - 1),
                    )
                    if mi == N_HB - 1:
                        dst, bias = (qT, bqT) if w_name == "q" else (kT, bkT)
                        nc.vector.tensor_scalar_add(
                            out=dst[:, nxt, n * QM:(n + 1) * QM],
                            in0=pstate["tile"],
                            scalar1=bias[:, nxt:nxt + 1],
                        )
                        pstate["ci"] += 1
                        pstate["mi"] = 0
                    else:
                        pstate["mi"] = mi + 1
                sc = ps_sc.tile([P, 2 * QM], FP32, tag="sc",
                                name=f"sc{hp}{qm}{kc}")
                for hh in range(2):
                    lo = hh * HD
                    nc.tensor.matmul(
                        sc[:, hh * QM:(hh + 1) * QM],
                        lhsT=kT[lo:lo + HD, hp, kc * P:(kc + 1) * P],
                        rhs=qT[lo:lo + HD, hp, qm * QM:(qm + 1) * QM],
                        start=True,
                        stop=True,
                        tile_position=(lo, 0),
                    )
            # flat [P, 1024] APs for the exp (a multi-free-dim AP can cost
            # extra ACT init cycles)
            et = expp.tile([P, 2 * QM], FP16, tag="exp")
            nc.scalar.activation(
                out=et, in_=sc,
                func=mybir.ActivationFunctionType.Exp,
                scale=0.125,
            )
            for hh in range(2):
                nc.tensor.matmul(
                    ctx_ps[hh],
                    lhsT=vp[:, kc, 2 * hp + hh, :],
                    rhs=et[:, hh * QM:(hh + 1) * QM],
                    start=(kc == 0),
                    stop=(kc == N_KC - 1),
                )

        def epilogue_a(ctx_ps, use_pe, ep_idx):
            """Drain the ctx accumulators out of PSUM and kick off the
            transposes. Emitted at the period boundary so the PSUM slots
            free quickly for the next period's accumulation."""
            tfulls = []
            for hh in range(2):
                csb = epil.tile([PD, QM], FP16, tag="ctxsb")
                if ep_idx < 2:
                    # rows 65:PD feed the xbar transpose as padding; each
                    # of the 4 ring buffers only needs zeroing once.
                    nc.vector.memset(csb[64:PD, :], 0.0)
                nc.vector.tensor_copy(out=csb[0:HD + 1, :], in_=ctx_ps[hh])
                if use_pe:
                    tp = ps_pj.tile([P, QM // P, HD + 2], FP16, tag="pj",
                                    name=f"tp{hh}")
                    for qs in range(QM // P):
                        nc.tensor.transpose(
                            tp[:, qs, 0:HD + 1],
                            csb[0:HD + 1, qs * P:(qs + 1) * P],
                            ident_b,
                        )
                    tfull = epil.tile([P, QM // P, HD + 1], FP16, tag="tpe")
                    nc.vector.tensor_copy(out=tfull, in_=tp[:, :, 0:HD + 1])
                else:
                    tfull = epil.tile([P, QM // P, PD], FP16, tag="tpsb")
                    for qs in range(QM // P):
                        nc.sync.dma_start_transpose(
                            out=tfull[:, qs, :],
                            in_=csb[:, qs * P:(qs + 1) * P],
                        )
                tfulls.append(tfull)
            return tfulls

        def epilogue_b(hp, qm, tfulls):
            """Reciprocal + scale + store. Deferred one period so these DVE
            ops are emitted after the next period's proj bias-adds and can
            never head-of-line-block them (the transposes they read were
            issued a full period earlier and are long done)."""
            for hh in range(2):
                tfull = tfulls[hh]
                rc = outp.tile([P, QM // P], FP32, tag="recip")
                nc.vector.reciprocal(out=rc, in_=tfull[:, :, HD:HD + 1])
                ot = outp.tile([P, QM // P, HD], FP32, tag="out")
                for qs in range(QM // P):
                    nc.vector.tensor_scalar_mul(
                        ot[:, qs, :], tfull[:, qs, 0:HD], rc[:, qs:qs + 1])
                row = qm * QM
                col = (2 * hp + hh) * HD
                nc.sync.dma_start(
                    out=out_d.ap()[row:row + QM, col:col + HD].rearrange(
                        "(a p) c -> p a c", p=P),
                    in_=ot,
                )

        phase1(None, None)
        pending = None
        pstate = make_proj_state()
        for hp in range(N_MT):
            for qm in range(N_QM):
                ctx_ps = new_ctx_ps(hp, qm)
                for kc in range(N_KC):
                    attn_kc(hp, qm, kc, ctx_ps, pstate)
                ep_idx = hp * N_QM + qm
                tfulls = epilogue_a(
                    ctx_ps,
                    use_pe=(hp == N_MT - 1 and qm == N_QM - 1),
                    ep_idx=ep_idx,
                )
                if pending is not None:
                    epilogue_b(*pending)
                pending = (hp, qm, tfulls)
        epilogue_b(*pending)


_NC_CACHE = None


def _get_nc():
    global _NC_CACHE
    if _NC_CACHE is None:
        _NC_CACHE = build()
    return _NC_CACHE


def make_in_maps(hidden_states, Wq, bq, Wk, bk, Wv, bv):
    hs = np.ascontiguousarray(np.asarray(hidden_states, dtype=np.float32))
    ws = {k: np.asarray(v, dtype=np.float32)
          for k, v in (("q", Wq), ("k", Wk), ("v", Wv))}
    bs = {k: np.asarray(v, dtype=np.float32)
          for k, v in (("q", bq), ("k", bk), ("v", bv))}
    in_maps = []
    for c in range(NCORES):
        b, g = c // 2, c % 2
        sl = slice(g * HG, (g + 1) * HG)
        in_maps.append({
            "x": np.ascontiguousarray(hs[b]),
            "wq": np.ascontiguousarray(ws["q"][:, sl]),
            "wk": np.ascontiguousarray(ws["k"][:, sl]),
            "wv": np.ascontiguousarray(ws["v"][:, sl]),
            "bq": np.ascontiguousarray(bs["q"][sl]),
            "bk": np.ascontiguousarray(bs["k"][sl]),
            "bv": np.ascontiguousarray(bs["v"][sl]),
        })
    return in_maps


def run(in_maps, trace=False):
    _ensure_profile_hook()
    nc = _get_nc()
    return run_bass_kernel_spmd(nc, in_maps, list(range(NCORES)), trace=trace)


def kernel(hidden_states, Wq, bq, Wk, bk, Wv, bv):
    in_maps = make_in_maps(hidden_states, Wq, bq, Wk, bk, Wv, bv)
    res = run(in_maps, trace=False)
    out = np.empty((B, S, H), dtype=np.float32)
    for c in range(NCORES):
        b, g = c // 2, c % 2
        out[b, :, g * HG:(g + 1) * HG] = res.results[c]["out"]
    return out
